# revision 1
# baseline (speedup 1.0000x reference)
"""Fused self-attention flow kernel for Trainium2 (8 NeuronCores).

Problem (hardcoded shapes): B=4, C=256, H=W=64, N=H*W=4096.
  x      = (inp [B,C,H,W] -> [B,N,C]) @ W_lin.T + b_lin
  scores = (x/16) @ x.T            # [B,N,N]
  attn   = softmax(scores, -1)
  out    = (attn @ flow [B,N,2]) -> [B,2,H,W]

Sharding: core c in 0..7 handles batch b=c//2, q-rows half=c%2.
Each core receives its batch's full inp (needed for K) *rolled* along N so
its own q-half occupies local rows 0..2047 -- every core runs the identical
SPMD program.

Per-core device program:
  1. xT[c_out, n] = W @ inp + b           (PE fp16 matmul + DVE bias-add)
  2. for each k-block (32 x 128):
       scoresT[k,q] (q=0..2047) in PSUM   (PE, accumulate over 2 C-chunks)
       probsT = exp(scoresT/16) -> SBUF   (ScalarE; no max-subtraction:
                                           scores <= ||x||^2/16 ~ 8 << 88)
       acc[3, q] += [f0,f1,1]^T @ probsT  (PE, 4-way column-tiled, PSUM acc)
  3. acc -> SBUF -> HBM "out" [12, 512]
     (col-group j holds q in [512j, 512j+512) on partitions 32j..32j+2)

Host: out[q,0:2] = acc[0:2,q]/acc[2,q], then unshard/reshape.
"""

import numpy as np

B, C, H, W = 4, 256, 64, 64
N = H * W          # 4096
QL = N // 2        # per-core q rows (2048)
KB = N // 128      # 32 k-blocks
NCORES = 8

_CACHE = {}


def _build_body(nc, tc, ctx, mybir, dram, fp8=False, fp8c=False):
    f32 = mybir.dt.float32
    f16 = mybir.dt.float16
    inp_d, wt_d, bias_d, flow3_d, out_d = dram

    sb = ctx.enter_context(tc.tile_pool(name="sb", bufs=1))
    probs_pool = ctx.enter_context(tc.tile_pool(name="probs", bufs=4))
    lin_ps = ctx.enter_context(tc.tile_pool(name="lin_ps", bufs=2, space="PSUM"))
    sc_ps = ctx.enter_context(tc.tile_pool(name="sc_ps", bufs=2, space="PSUM"))
    out_ps_pool = ctx.enter_context(tc.tile_pool(name="out_ps", bufs=1, space="PSUM"))

    # --- constants / small inputs ---
    wt_sb = sb.tile([128, 2, C], f16)
    nc.sync.dma_start(out=wt_sb[:], in_=wt_d[:])
    bias_sb = sb.tile([128, 2], f32)
    nc.sync.dma_start(out=bias_sb[:], in_=bias_d[:])
    flow3_sb = sb.tile([128, KB, 3], f16)
    nc.sync.dma_start(out=flow3_sb[:], in_=flow3_d[:])

    # exp bias: exp(s/16 - 4) — cancels in the softmax ratio, keeps fp16
    # probs far from overflow
    exp_bias = sb.tile([128, 1], f32)
    nc.vector.memset(exp_bias[:], -4.0)

    # warm up the exp table-load (~2.7us) under the input DMA
    warm = sb.tile([128, 8], f32)
    nc.vector.memset(warm[:], 0.0)
    nc.scalar.activation(out=warm[:], in_=warm[:],
                         func=mybir.ActivationFunctionType.Exp)

    # --- inp DMA (n-tile major so the linear can start early; fp16 from host)
    # --- linear xT[oc*128+p, n] = sum_ic W.T[ic, oc] . inp[ic] + b
    f8 = mybir.dt.float8e4
    inp_f16 = [sb.tile([128, N], f16, name=f"inp_f16_{ic}", tag=f"inpf{ic}")
               for ic in range(2)]
    if fp8:
        xT8 = sb.tile([128, 2, N], f8, name="xT8", tag="xT8")
    elif fp8c:
        xT = [sb.tile([128, N], f16, name=f"xT{oc}", tag=f"xT{oc}")
              for oc in range(2)]
        xT8 = sb.tile([128, 2, N], f8, name="xT8", tag="xT8")
        eT8 = sb.tile([128, 2, N], f8, name="eT8", tag="eT8")
    else:
        xT = [sb.tile([128, N], f16, name=f"xT{oc}", tag=f"xT{oc}")
              for oc in range(2)]
    for nt in range(8):
        s = slice(nt * 512, (nt + 1) * 512)
        for ic in range(2):
            nc.sync.dma_start(out=inp_f16[ic][:, s],
                              in_=inp_d[ic * 128:(ic + 1) * 128, s])
        for oc in range(2):
            pl = lin_ps.tile([128, 512], f32, name="pl", tag="pl")
            for ic in range(2):
                nc.tensor.matmul(
                    pl[:],
                    lhsT=wt_sb[:, ic, oc * 128:(oc + 1) * 128],
                    rhs=inp_f16[ic][:, s],
                    start=(ic == 0), stop=(ic == 1),
                )
            dst = xT8[:, oc, s] if fp8 else xT[oc][:, s]
            nc.vector.tensor_scalar_add(dst, pl[:], bias_sb[:, oc:oc + 1])
            if fp8c:
                # x8 = fp8(x16); e8 = fp8(x16 - x8)  (first-order correction)
                nc.vector.tensor_copy(out=xT8[:, oc, s], in_=xT[oc][:, s])
                nc.vector.tensor_sub(eT8[:, oc, s], xT[oc][:, s], xT8[:, oc, s])

    # --- attention over local q rows 0..QL, all 4096 k ---
    out_acc = out_ps_pool.tile([128, 512], f32)

    def emit_pf(kb, pts):
        # 4 skinny accumulating matmuls back-to-back -> 4-way column-tiled
        # concurrency in the PE array
        for qt in range(2):
            for nn in range(2):
                j = qt * 2 + nn  # column-group / q-subtile 0..3
                nc.tensor.matmul(
                    out_acc[32 * j:32 * j + 3, :],
                    lhsT=flow3_sb[:, kb, :],
                    rhs=pts[qt][:, nn * 512:(nn + 1) * 512],
                    start=(kb == 0), stop=(kb == KB - 1),
                    tile_position=(0, 32 * j),
                    skip_group_check=True,
                )

    pending = None  # defer each kb's P@F past kb+1's score matmuls
    for kb in range(KB):
        ks = slice(kb * 128, (kb + 1) * 128)
        pts = []
        for qt in range(2):  # two 1024-wide q sub-tiles
            ps = sc_ps.tile([128, 1024], f32, name="ps", tag="ps")
            for nn in range(2):
                qs = slice(qt * 1024 + nn * 512, qt * 1024 + (nn + 1) * 512)
                if fp8:
                    nc.tensor.matmul(
                        ps[:, nn * 512:(nn + 1) * 512],
                        lhsT=xT8[:, :, ks],
                        rhs=xT8[:, :, qs],
                        start=True, stop=True,
                        perf_mode=mybir.MatmulPerfMode.DoubleRow,
                    )
                elif fp8c:
                    dst = ps[:, nn * 512:(nn + 1) * 512]
                    for i, (a, bt) in enumerate(
                            ((xT8, xT8), (xT8, eT8), (eT8, xT8))):
                        nc.tensor.matmul(
                            dst, lhsT=a[:, :, ks], rhs=bt[:, :, qs],
                            start=(i == 0), stop=(i == 2),
                            perf_mode=mybir.MatmulPerfMode.DoubleRow,
                        )
                else:
                    for ic in range(2):
                        nc.tensor.matmul(
                            ps[:, nn * 512:(nn + 1) * 512],
                            lhsT=xT[ic][:, ks],
                            rhs=xT[ic][:, qs],
                            start=(ic == 0), stop=(ic == 1),
                        )
            pt = probs_pool.tile([128, 1024], f16, name="pt", tag="pt")
            # exp(s/16 - 4): the constant shift cancels in the softmax ratio
            # (host divides num by den) and keeps fp16 probs far from overflow
            nc.scalar.activation(out=pt[:], in_=ps[:],
                                 func=mybir.ActivationFunctionType.Exp,
                                 scale=float(C) ** -0.5, bias=exp_bias[:])
            pts.append(pt)
        if pending is not None:
            emit_pf(*pending)
        pending = (kb, pts)
    emit_pf(*pending)

    out_sb = sb.tile([128, 512], f32)
    for j in range(4):
        nc.vector.tensor_copy(out=out_sb[32 * j:32 * j + 3, :],
                              in_=out_acc[32 * j:32 * j + 3, :])
        nc.sync.dma_start(out=out_d[j], in_=out_sb[32 * j:32 * j + 3, :])


def _build_nc(reps=1, fp8=False, fp8c=False):
    from contextlib import ExitStack

    import concourse.bacc as bacc
    import concourse.tile as tile
    from concourse import mybir

    f32 = mybir.dt.float32
    f16 = mybir.dt.float16

    nc = bacc.Bacc("TRN2", target_bir_lowering=False, debug=False)

    dram = (
        nc.dram_tensor("inp", (C, N), f16, kind="ExternalInput"),
        nc.dram_tensor("wt", (128, 2, C), f16, kind="ExternalInput"),
        nc.dram_tensor("bias", (128, 2), f32, kind="ExternalInput"),
        nc.dram_tensor("flow3", (128, KB, 3), f16, kind="ExternalInput"),
        nc.dram_tensor("out", (4, 3, 512), f32, kind="ExternalOutput"),
    )

    with tile.TileContext(nc) as tc:
        for _ in range(reps):
            with ExitStack() as ctx:
                _build_body(nc, tc, ctx, mybir, dram, fp8=fp8,
                            fp8c=fp8c)

    nc.compile()
    return nc


import os
_FP8 = os.environ.get("K_FP8", "0") == "1"
_FP8C = os.environ.get("K_FP8C", "0") == "1"


def _get_nc(reps=1):
    key = ("nc", reps, _FP8, _FP8C)
    if key not in _CACHE:
        _CACHE[key] = _build_nc(reps, fp8=_FP8, fp8c=_FP8C)
    return _CACHE[key]


def _make_in_maps(inp, flow_init, W_lin, b_lin):
    inp = np.ascontiguousarray(np.asarray(inp, dtype=np.float32)).reshape(B, C, N)
    flow = np.ascontiguousarray(np.asarray(flow_init, dtype=np.float32)).reshape(B, 2, N)
    W_lin = np.asarray(W_lin, dtype=np.float32)
    b_lin = np.asarray(b_lin, dtype=np.float32)

    # lhsT layout for xT = W @ inp: [c_in(part 128), ic, c_out]
    wt = np.ascontiguousarray(
        W_lin.T.reshape(2, 128, C).transpose(1, 0, 2)).astype(np.float16)
    bias = np.ascontiguousarray(b_lin.reshape(2, 128).T)        # [128, 2]

    in_maps = []
    for c in range(NCORES):
        b, half = divmod(c, 2)
        sh = -QL * half
        inp_c = np.roll(inp[b], sh, axis=1) if half else inp[b]
        inp_c = inp_c.astype(np.float16)
        f = np.roll(flow[b], sh, axis=1) if half else flow[b]
        flow3 = np.empty((N, 3), np.float16)
        flow3[:, 0:2] = f.T
        flow3[:, 2] = 1.0
        flow3_c = np.ascontiguousarray(
            flow3.reshape(KB, 128, 3).transpose(1, 0, 2))       # [128, KB, 3]
        in_maps.append({
            "inp": np.ascontiguousarray(inp_c),
            "wt": wt,
            "bias": bias,
            "flow3": flow3_c,
        })
    return in_maps


def _postprocess(results):
    out = np.empty((B, 2, N), np.float32)
    for c in range(NCORES):
        b, half = divmod(c, 2)
        acc = results[c]["out"].reshape(12, 512)
        groups = [acc[3 * j:3 * j + 3, :] for j in range(4)]
        a = np.concatenate(groups, axis=1)                       # [3, 2048]
        out[b, :, half * QL:(half + 1) * QL] = a[0:2] / a[2]
    return out.reshape(B, 2, H, W)


def _run(inputs, trace=False):
    from concourse.bass_utils import run_bass_kernel_spmd

    nc = _get_nc()
    in_maps = _make_in_maps(inputs["inp"], inputs["flow_init"],
                            inputs["W_lin"], inputs["b_lin"])
    r = run_bass_kernel_spmd(nc, in_maps, core_ids=list(range(NCORES)),
                             trace=False)
    _CACHE["last_exec_ns"] = r.exec_time_ns
    return _postprocess(r.results)


def kernel(**inputs) -> np.ndarray:
    return _run(inputs, trace=False)



# revision 7
# speedup vs baseline: 1.0315x; 1.0315x over previous
"""Fused self-attention flow kernel for Trainium2 (8 NeuronCores), v2.

Problem (hardcoded): B=4, C=256, H=W=64, N=4096.
  x      = (inp [B,C,H,W] -> [B,N,C]) @ W_lin.T + b_lin
  scores = (x/16) @ x.T
  attn   = softmax(scores, -1)
  out    = (attn @ flow [B,N,2]) -> [B,2,H,W]

Sharding: core c handles batch c//2, q-half c%2; inputs rolled along N so
local q rows are 0..2047 (SPMD-identical program on all cores).

Device algorithm (per core):
  1. Linear in fp16 (PE), bias added via a 1-partition fp8 ones-matmul into
     the same PSUM accumulation; PSUM -> two fp8 limbs xh=fp8(x),
     xl=fp8(x-xh) (DVE/Pool).
  2. Scores in fp8 DoubleRow (2 rows/cycle): s = xh_k . xh_q, one [128,2,512]
     PSUM tile per (k-pair j of 256, q-chunk c of 512). On the 8 diagonal
     windows (j<8, c=j//2) two extra correction matmuls add
     xl_k.xh_q + xh_k.xl_q, fixing the |x_q|^2 diagonal whose correlated
     quantization error otherwise dominates (softmax rows with large
     diagonal weight).
  3. exp(s/16 - 1) split across three engines:
       ACT:  true exp (table), fp16 out on diag tiles, fp8 out elsewhere
       DVE/Pool: Schraudolph fast-exp -- one tensor_scalar
                 (s*MUL + ADD -> int8) whose integer result IS the fp8 bit
                 pattern (bitcast); max rel err ~3%, which averages out in
                 the softmax ratio (validated: 6.8e-3 end-to-end).
     The -1 shift cancels in num/den.
  4. attn @ [f0,f1,1]: fp8 DoubleRow over k-pairs (non-diag) and fp16 over
     single k-blocks (diag), accumulated in one PSUM bank via 4-way
     column-tiled groups (one 32-col group per q-chunk).
  5. Host divides num/den and unshards.
"""

import os

import numpy as np

B, C, H, W = 4, 256, 64, 64
N = H * W            # 4096
QL = N // 2          # per-core q rows
PAIRS = 16           # k-pairs of 256
CHUNKS = 4           # q chunks of 512
NCORES = 8

SHIFT = -1.0
_LN2 = float(np.log(2.0))
MUL8 = 0.0625 * 8.0 / _LN2
ADD8 = 56.0 - 0.344 + SHIFT * 8.0 / _LN2
MUL16 = 0.0625 * 1024.0 / _LN2
ADD16 = 15360.0 - 44.0 + SHIFT * 1024.0 / _LN2

_CACHE = {}

# engine schedule for the 56 non-diag exp tiles (diag tiles always ACT/fp16)
# counts tuned for rate balance: ACT 1029ns/tile, Pool 889, DVE 1174
_N_ACT = int(os.environ.get("K_NACT", "25"))
_N_POOL = int(os.environ.get("K_NPOOL", "0"))
_CONV_POOL = int(os.environ.get("K_CONVPOOL", "22"))  # of 32 conv ops


def _mk_schedule():
    """(j, c) -> 'act16' | 'act' | 'dve' | 'pool' for the 64 exp tiles."""
    sched = {}
    order = [(j, c) for j in range(PAIRS) for c in range(CHUNKS)]
    nondiag = [t for t in order if not (t[0] < 8 and t[1] == t[0] // 2)]
    n = len(nondiag)
    quota = {"act": _N_ACT, "pool": _N_POOL, "dve": n - _N_ACT - _N_POOL}
    filled = {"act": 0, "pool": 0, "dve": 0}
    for i, t in enumerate(nondiag):
        # largest remaining deficit, interleaved
        best = max(quota, key=lambda e: quota[e] - filled[e] * n / max(1, quota[e]) if quota[e] else -1e9)
        # simpler: proportional position
        best, bestv = None, -1e9
        for e in ("act", "pool", "dve"):
            if quota[e] == 0:
                continue
            v = quota[e] * (i + 1) / n - filled[e]
            if v > bestv:
                best, bestv = e, v
        sched[t] = best
        filled[best] += 1
    for j in range(8):
        sched[(j, j // 2)] = "act16"
    return sched


def _build_body(nc, tc, ctx, mybir, dram):
    f32 = mybir.dt.float32
    f16 = mybir.dt.float16
    f8 = mybir.dt.float8e4
    i16 = mybir.dt.int16
    inp_d, wt_d, b8_d, flow16_d, out_d = dram

    sb = ctx.enter_context(tc.tile_pool(name="sb", bufs=1))
    pt16_pool = ctx.enter_context(tc.tile_pool(name="pt16", bufs=6))
    lin_ps = ctx.enter_context(tc.tile_pool(name="lin_ps", bufs=2, space="PSUM"))
    sc_ps = ctx.enter_context(tc.tile_pool(name="sc_ps", bufs=2, space="PSUM"))
    out_ps_pool = ctx.enter_context(tc.tile_pool(name="out_ps", bufs=1, space="PSUM"))

    sched = _mk_schedule()

    # --- constants / small inputs ---
    wt_sb = sb.tile([128, 2, C], f16)
    nc.sync.dma_start(out=wt_sb[:], in_=wt_d[:])
    b8_sb = sb.tile([1, 2, C], f8)
    nc.sync.dma_start(out=b8_sb[:], in_=b8_d[:])
    flow16_sb = sb.tile([128, 32, 3], f16)
    nc.sync.dma_start(out=flow16_sb[:], in_=flow16_d[:])

    ones8 = sb.tile([1, 2, 512], f8)
    nc.vector.memset(ones8[:, 0, :], 1.0)
    nc.vector.memset(ones8[:, 1, :], 0.0)

    shift_sb = sb.tile([128, 1], f32)
    nc.vector.memset(shift_sb[:], SHIFT)

    # warm up the exp table-load (~1.3us) under the input DMA
    warm = sb.tile([128, 8], f32)
    nc.vector.memset(warm[:], 0.0)
    nc.scalar.activation(out=warm[:], in_=warm[:],
                         func=mybir.ActivationFunctionType.Exp)

    # --- phase A: linear + 2-limb fp8 quantization ---
    inp_f16 = [sb.tile([128, N], f16, name=f"inp_f16_{ic}", tag=f"inpf{ic}")
               for ic in range(2)]
    xh8 = sb.tile([128, 2, N], f8, name="xh8", tag="xh8")
    xl8 = sb.tile([128, 2, QL], f8, name="xl8", tag="xl8")

    conv_i = 0
    for nt in range(8):
        s = slice(nt * 512, (nt + 1) * 512)
        for ic in range(2):
            nc.sync.dma_start(out=inp_f16[ic][:, s],
                              in_=inp_d[ic * 128:(ic + 1) * 128, s])
        for oc in range(2):
            pl = lin_ps.tile([128, 512], f32, name="pl", tag="pl")
            for ic in range(2):
                nc.tensor.matmul(
                    pl[:],
                    lhsT=wt_sb[:, ic, oc * 128:(oc + 1) * 128],
                    rhs=inp_f16[ic][:, s],
                    start=(ic == 0), stop=False,
                )
            nc.tensor.matmul(
                pl[:],
                lhsT=b8_sb[:, :, oc * 128:(oc + 1) * 128],
                rhs=ones8[:],
                start=False, stop=True,
                perf_mode=mybir.MatmulPerfMode.DoubleRow,
            )
            nc.scalar.activation(out=xh8[:, oc, s], in_=pl[:],
                                 func=mybir.ActivationFunctionType.Copy,
                                 bias=0.0)
            if nt < 4:
                nc.vector.tensor_sub(xl8[:, oc, s], pl[:], xh8[:, oc, s])

    # --- phase B: scores + exp + PF ---
    out_acc = out_ps_pool.tile([128, 512], f32)
    group_seen = [0] * CHUNKS
    GROUP_TOTAL = PAIRS * 2  # per group: 2 fp16 matmuls per pair

    for j in range(PAIRS):
        pts = [None] * CHUNKS           # (fmt, tile)
        for c in range(CHUNKS):
            qs = slice(c * 512, (c + 1) * 512)
            diag = sched[(j, c)] == "act16"
            ps = sc_ps.tile([128, 2, 512], f32, name="ps", tag="ps")
            for i in range(2):
                ks = slice(j * 256 + i * 128, j * 256 + (i + 1) * 128)
                terms = [(xh8, xh8)]
                if diag:
                    terms += [(xl8, xh8), (xh8, xl8)]
                for ti, (lt, rt) in enumerate(terms):
                    nc.tensor.matmul(
                        ps[:, i, :],
                        lhsT=lt[:, :, ks],
                        rhs=rt[:, :, qs],
                        start=(ti == 0), stop=(ti == len(terms) - 1),
                        perf_mode=mybir.MatmulPerfMode.DoubleRow,
                    )
            eng = sched[(j, c)]
            pt = pt16_pool.tile([128, 2, 512], f16, name="pt16", tag="pt16")
            if eng in ("act16", "act"):
                nc.scalar.activation(out=pt[:], in_=ps[:],
                                     func=mybir.ActivationFunctionType.Exp,
                                     scale=0.0625, bias=shift_sb[:])
            else:
                e = nc.vector if eng == "dve" else nc.gpsimd
                e.tensor_scalar(
                    out=pt[:].bitcast(i16), in0=ps[:],
                    scalar1=MUL16, scalar2=ADD16,
                    op0=mybir.AluOpType.mult, op1=mybir.AluOpType.add,
                )
            pts[c] = pt

        # PF batch: two fp16 matmuls per chunk, 4-way col-tiled groups
        for c in range(CHUNKS):
            pt = pts[c]
            for i in range(2):
                group_seen[c] += 1
                nc.tensor.matmul(
                    out_acc[32 * c:32 * c + 3, :],
                    lhsT=flow16_sb[:, 2 * j + i, :],
                    rhs=pt[:, i, :],
                    start=(group_seen[c] == 1),
                    stop=(group_seen[c] == GROUP_TOTAL),
                    tile_position=(0, 32 * c),
                    skip_group_check=True,
                )

    out_sb = sb.tile([128, 512], f32)
    for c in range(CHUNKS):
        nc.vector.tensor_copy(out=out_sb[32 * c:32 * c + 3, :],
                              in_=out_acc[32 * c:32 * c + 3, :])
        nc.sync.dma_start(out=out_d[c], in_=out_sb[32 * c:32 * c + 3, :])


def _build_nc(reps=1, **_unused):
    from contextlib import ExitStack

    import concourse.bacc as bacc
    import concourse.tile as tile
    from concourse import mybir

    f32 = mybir.dt.float32
    f16 = mybir.dt.float16
    f8 = mybir.dt.float8e4

    nc = bacc.Bacc("TRN2", target_bir_lowering=False, debug=False)

    dram = (
        nc.dram_tensor("inp", (C, N), f16, kind="ExternalInput"),
        nc.dram_tensor("wt", (128, 2, C), f16, kind="ExternalInput"),
        nc.dram_tensor("b8", (1, 2, C), f8, kind="ExternalInput"),
        nc.dram_tensor("flow16", (128, 32, 3), f16, kind="ExternalInput"),
        nc.dram_tensor("out", (4, 3, 512), f32, kind="ExternalOutput"),
    )

    with tile.TileContext(nc) as tc:
        for _ in range(reps):
            with ExitStack() as ctx:
                _build_body(nc, tc, ctx, mybir, dram)

    nc.compile()
    return nc


_FP8 = False   # kept for bench_hw compat
_FP8C = False


def _get_nc(reps=1):
    key = ("nc", reps)
    if key not in _CACHE:
        _CACHE[key] = _build_nc(reps)
    return _CACHE[key]


def _make_in_maps(inp, flow_init, W_lin, b_lin):
    import ml_dtypes
    F8 = ml_dtypes.float8_e4m3

    inp = np.ascontiguousarray(np.asarray(inp, dtype=np.float32)).reshape(B, C, N)
    flow = np.ascontiguousarray(np.asarray(flow_init, dtype=np.float32)).reshape(B, 2, N)
    W_lin = np.asarray(W_lin, dtype=np.float32)
    b_lin = np.asarray(b_lin, dtype=np.float32)

    # lhsT layout for x = W @ inp: [c_in(part 128), ic, c_out]
    wt = np.ascontiguousarray(
        W_lin.T.reshape(2, 128, C).transpose(1, 0, 2)).astype(np.float16)
    b8 = np.zeros((1, 2, C), F8)
    b8[0, 0, :] = b_lin.astype(F8)

    in_maps = []
    for c in range(NCORES):
        b, half = divmod(c, 2)
        sh = -QL * half
        inp_c = np.roll(inp[b], sh, axis=1) if half else inp[b]
        inp_c = inp_c.astype(np.float16)
        f = np.roll(flow[b], sh, axis=1) if half else flow[b]
        flow3 = np.empty((N, 3), np.float32)
        flow3[:, 0:2] = f.T
        flow3[:, 2] = 1.0
        flow16 = np.ascontiguousarray(
            flow3.reshape(32, 128, 3).transpose(1, 0, 2)).astype(np.float16)
        in_maps.append({
            "inp": np.ascontiguousarray(inp_c),
            "wt": wt,
            "b8": b8,
            "flow16": flow16,
        })
    return in_maps


def _postprocess(results):
    out = np.empty((B, 2, N), np.float32)
    for c in range(NCORES):
        b, half = divmod(c, 2)
        acc = results[c]["out"].reshape(12, 512)
        groups = [acc[3 * j:3 * j + 3, :] for j in range(4)]
        a = np.concatenate(groups, axis=1)                       # [3, 2048]
        out[b, :, half * QL:(half + 1) * QL] = a[0:2] / a[2]
    return out.reshape(B, 2, H, W)


def _run(inputs, trace=False):
    from concourse.bass_utils import run_bass_kernel_spmd

    nc = _get_nc()
    in_maps = _make_in_maps(inputs["inp"], inputs["flow_init"],
                            inputs["W_lin"], inputs["b_lin"])
    r = run_bass_kernel_spmd(nc, in_maps, core_ids=list(range(NCORES)),
                             trace=False)
    _CACHE["last_exec_ns"] = r.exec_time_ns
    return _postprocess(r.results)


def kernel(**inputs) -> np.ndarray:
    return _run(inputs, trace=False)
